# revision 4
# baseline (speedup 1.0000x reference)
"""Trainium2 Bass kernel for nn_Attention_14877766713476.

Causal multi-head attention with full-hidden RoPE:
  q,k,v = x@W{q,k,v} + b;  q,k = rope(q,k);  heads=16, hd=128;
  causal softmax attention;  out = attn@Wo + bo.

Sharding: tensor-parallel over heads across 8 cores. RoPE pairs hidden
column c with c +/- 1024, i.e. head h with head h+8 -- so core m owns
heads {m, m+8} and RoPE stays core-local. Each core computes its two
heads end-to-end and a partial output projection (rows of Wo); the host
sums the 8 partials.

All matmuls in bf16 with fp32 PSUM accumulation. Host pre-transposes
x -> xT (contraction dim on partitions) and pre-slices/casts weights,
so the device does zero transposes.

v2 changes over the baseline:
  - biases dropped on device (spec fills them with zeros; host numpy
    fallback covers the general case), raw q/k evicted via ScalarE.
  - diagonal score blocks compute only their unmasked column range
    (scores, exp and PV all shrink); the per-block causal triangle is
    a single [128,128] mask on GpSimd; masked columns of the exp tile
    are zeroed by small GpSimd memsets (for the rowsum).
  - attention inner loop is pipelined 2 pairs deep so PE never waits
    on ScalarE's exp.
  - exp-rowsum presums split between DVE and GpSimd.
  - prologue DMA order interleaves wq/x groups on the sync ring while
    wk/wv/cos/sin stream on the gpsimd ring.

Layouts (per core, host-prepared, all bf16 unless noted):
  xT    [128, 16*4096]  col = a*4096 + t   (d = a*128 + p, t = b*2048+s)
  wq/wk/wv [128, 16*256] col = a*256 + c   (d = a*128 + p, c in 0..255)
  wo    [128, 2*2048]   col = cb*2048 + dcol  (c = cb*128 + p)
  cosT/sinT [128, 2*4096] col = cb*4096 + t   (c = cb*128 + p; sinT block0
                           negated so rope_b = q_b*cos_b + q_{1-b}*sinT_b)
  tri   [128, 128]      tri[kj, qq] = (qq >= kj)  (intra-block causal)
  ones  [128, 128]      all ones (rowsum matmul stationary)
Output per core: out [4096, 2048] bf16 partial (this core's two heads
through Wo rows); host sums partials in fp32 and adds bv@Wo + bo.
"""

import math
from contextlib import ExitStack

import numpy as np
import ml_dtypes

N_CORES = 8
B, S, D, H = 2, 2048, 2048, 16
HD = D // H          # 128
T = B * S            # 4096
P = 128
NB = D // P          # 16 contraction blocks
NG = 4               # a-groups (DMA split granularity)
GA = NB // NG        # 4 a-blocks per group
TCH = 512            # token chunk (QKV phase free dim)
NCH = T // TCH       # 8
QBLK = 512           # query block (attention phase free dim)
NQ = S // QBLK       # 4 query blocks per (batch, head)
SCALE = 1.0 / math.sqrt(HD)

BF16 = ml_dtypes.bfloat16

_CACHE = {}
LAST_RESULTS = None


def _build_program():
    import concourse.tile as tile
    from concourse import bacc, mybir

    bf = mybir.dt.bfloat16
    f32 = mybir.dt.float32
    Act = mybir.ActivationFunctionType

    nc = bacc.Bacc("TRN2", target_bir_lowering=False, debug=False,
                   num_devices=N_CORES)

    xT = nc.dram_tensor("xT", [P, NB * T], bf, kind="ExternalInput").ap()
    wq = nc.dram_tensor("wq", [P, NB * 256], bf, kind="ExternalInput").ap()
    wk = nc.dram_tensor("wk", [P, NB * 256], bf, kind="ExternalInput").ap()
    wv = nc.dram_tensor("wv", [P, NB * 256], bf, kind="ExternalInput").ap()
    wo = nc.dram_tensor("wo", [P, 2 * D], bf, kind="ExternalInput").ap()
    cosT = nc.dram_tensor("cosT", [P, 2 * T], bf, kind="ExternalInput").ap()
    sinT = nc.dram_tensor("sinT", [P, 2 * T], bf, kind="ExternalInput").ap()
    tri = nc.dram_tensor("tri", [P, P], bf, kind="ExternalInput").ap()
    ones = nc.dram_tensor("ones", [P, P], bf, kind="ExternalInput").ap()
    out = nc.dram_tensor("out", [T, D], bf, kind="ExternalOutput").ap()

    xT4 = xT.rearrange("p (g a t) -> p g a t", g=NG, a=GA)
    wq4 = wq.rearrange("p (g a c) -> p g a c", g=NG, a=GA)
    wk4 = wk.rearrange("p (g a c) -> p g a c", g=NG, a=GA)
    wv4 = wv.rearrange("p (g a c) -> p g a c", g=NG, a=GA)
    cosT3 = cosT.rearrange("p (c t) -> p c t", c=2)
    sinT3 = sinT.rearrange("p (c t) -> p c t", c=2)

    with tile.TileContext(nc) as tc, ExitStack() as ctx:
        const = ctx.enter_context(tc.tile_pool(name="const", bufs=1))
        persist = ctx.enter_context(tc.tile_pool(name="persist", bufs=1))
        xt_pool = ctx.enter_context(tc.tile_pool(name="xt", bufs=2))
        cs_pool = ctx.enter_context(tc.tile_pool(name="cs", bufs=2))
        raw_pool = ctx.enter_context(tc.tile_pool(name="raw", bufs=2))
        tmp_pool = ctx.enter_context(tc.tile_pool(name="tmp", bufs=4))
        exp_pool = ctx.enter_context(tc.tile_pool(name="exp", bufs=5))
        rec_pool = ctx.enter_context(tc.tile_pool(name="rec", bufs=2))
        orow_pool = ctx.enter_context(tc.tile_pool(name="orow", bufs=2))

        # Weight/x prologue: interleave (wq_g, xt_g) pairs on the sync
        # ring so the q accumulation chain never outruns its weights;
        # wk/wv/cos/sin stream in parallel on the gpsimd ring, ordered
        # by first use (k matmuls, v matmuls, then RoPE).
        wq_sb = [const.tile([P, GA, 256], bf, tag=f"wq{g}", name=f"wq_sb{g}")
                 for g in range(NG)]
        wk_sb = [const.tile([P, GA, 256], bf, tag=f"wk{g}", name=f"wk_sb{g}")
                 for g in range(NG)]
        wv_sb = [const.tile([P, GA, 256], bf, tag=f"wv{g}", name=f"wv_sb{g}")
                 for g in range(NG)]
        xt0 = [xt_pool.tile([P, GA, TCH], bf, tag=f"xt{g}", name=f"xt{g}_0")
               for g in range(NG)]
        nc.sync.dma_start(wq_sb[0][:], wq4[:, 0])
        nc.sync.dma_start(xt0[0][:, 0:2, :], xT4[:, 0, 0:2, 0:TCH])
        nc.sync.dma_start(xt0[0][:, 2:4, :], xT4[:, 0, 2:4, 0:TCH])
        for g in range(1, NG):
            nc.sync.dma_start(wq_sb[g][:], wq4[:, g])
            nc.sync.dma_start(xt0[g][:], xT4[:, g, :, 0:TCH])
        for g in range(NG):
            nc.gpsimd.dma_start(wk_sb[g][:], wk4[:, g])
        for g in range(NG):
            nc.gpsimd.dma_start(wv_sb[g][:], wv4[:, g])

        cos0 = cs_pool.tile([P, 2, TCH], bf, tag="cos", name="cosc_0")
        nc.gpsimd.dma_start(cos0[:], cosT3[:, :, 0:TCH])
        sin0 = cs_pool.tile([P, 2, TCH], bf, tag="sin", name="sinc_0")
        nc.gpsimd.dma_start(sin0[:], sinT3[:, :, 0:TCH])

        wo_sb = const.tile([P, 2 * D], bf, tag="wo")
        tri_sb = const.tile([P, P], bf, tag="tri")
        ones_sb = const.tile([P, P], bf, tag="ones")
        nc.gpsimd.dma_start(tri_sb[:], tri[:])
        nc.gpsimd.dma_start(ones_sb[:], ones[:])
        nc.gpsimd.dma_start(wo_sb[:], wo[:])

        # persistent activations
        q_all = persist.tile([P, 2 * T], bf, tag="q_all")      # roped qT
        k_all = persist.tile([P, 2 * T], bf, tag="k_all")      # roped kT
        v_all = persist.tile([P, 32 * 256], bf, tag="v_all")   # v natural
        at_all = persist.tile([P, 2 * T], bf, tag="at_all")    # attnT

        # ---------------- Phase 1: QKV projections + RoPE ----------------
        with tc.tile_pool(name="psum1", bufs=3, space="PSUM") as psum:
            for tcix in range(NCH - 1):
                t0 = tcix * TCH
                if tcix == 0:
                    xt = xt0
                    cosc, sinc = cos0, sin0
                else:
                    xt = [xt_pool.tile([P, GA, TCH], bf, tag=f"xt{g}",
                                       name=f"xt{g}_{tcix}")
                          for g in range(NG)]
                    for g in range(NG):
                        nc.sync.dma_start(xt[g][:], xT4[:, g, :, t0:t0 + TCH])
                    cosc = cs_pool.tile([P, 2, TCH], bf, tag="cos")
                    nc.sync.dma_start(cosc[:], cosT3[:, :, t0:t0 + TCH])
                    sinc = cs_pool.tile([P, 2, TCH], bf, tag="sin")
                    nc.sync.dma_start(sinc[:], sinT3[:, :, t0:t0 + TCH])

                qraw = raw_pool.tile([P, 2, TCH], bf, tag="qraw")
                kraw = raw_pool.tile([P, 2, TCH], bf, tag="kraw")
                for (wt, rawt) in ((wq_sb, qraw), (wk_sb, kraw)):
                    for cb in range(2):
                        ps = psum.tile([P, TCH], f32, tag="qk")
                        for a in range(NB):
                            nc.tensor.matmul(
                                ps[:],
                                wt[a // GA][:, a % GA,
                                            cb * P:cb * P + P],
                                xt[a // GA][:, a % GA, :],
                                start=(a == 0), stop=(a == NB - 1),
                            )
                        nc.scalar.activation(rawt[:, cb, :], ps[:], Act.Copy)
                # v: x-stationary, natural layout
                for tt in range(TCH // P):
                    ps = psum.tile([P, 256], f32, tag="v")
                    for a in range(NB):
                        nc.tensor.matmul(
                            ps[:],
                            xt[a // GA][:, a % GA, tt * P:(tt + 1) * P],
                            wv_sb[a // GA][:, a % GA, :],
                            start=(a == 0), stop=(a == NB - 1),
                        )
                    cidx = (tcix * (TCH // P) + tt) * 256
                    nc.scalar.activation(v_all[:, cidx:cidx + 256], ps[:],
                                         Act.Copy)

                # RoPE: rope_b = raw_b*cos_b + raw_{1-b}*sinT_b (sign-folded)
                for (rawt, dst) in ((qraw, q_all), (kraw, k_all)):
                    for cb in range(2):
                        tm = tmp_pool.tile([P, TCH], bf, tag="ropetmp")
                        nc.vector.tensor_mul(tm[:], rawt[:, 1 - cb, :],
                                             sinc[:, cb, :])
                        tm2 = tmp_pool.tile([P, TCH], bf, tag="ropetmp2")
                        nc.vector.tensor_mul(tm2[:], rawt[:, cb, :],
                                             cosc[:, cb, :])
                        nc.vector.tensor_add(
                            dst[:, cb * T + t0:cb * T + t0 + TCH],
                            tm[:], tm2[:])

        # ------- Phase 2+3: causal attention + output projection -------
        # scoresT blocks [kj=128, q=512]; diagonal blocks shrink to their
        # unmasked column range; exp on ScalarE; per-block causal triangle
        # via a [128,128] GpSimd mask; rowsum via ones-matmul on quad
        # presums (DVE+GpSimd); PV consumes expT directly. The inner loop
        # is software-pipelined two pairs deep (rs/pv trail sc/exp by two
        # pairs) so PE never head-of-line blocks on ScalarE's exp. The
        # output projection for a (b, qj) token group is interleaved one
        # group late, once its at_all slices are long since written.
        def attn_group(psum, b, cb, qj):
            qs = cb * T + b * S + qj * QBLK
            nkb = 4 * qj + 4  # key blocks 0..nkb-1
            nquads = nkb // 4
            pv_ps = psum.tile([P, QBLK], f32, tag="pv", bufs=1,
                              name=f"pv_{b}{cb}{qj}")
            rs_ps = psum.tile([P, QBLK], f32, tag="rs", bufs=1,
                              name=f"rs_{b}{cb}{qj}")
            quad_buf = []

            def blk_lo(i):
                # first unmasked column of key block i within this q block
                dd = i - 4 * qj
                return 128 * dd if dd > 0 else 0

            def consume(ii, ex):
                for h in range(2):
                    i = 2 * ii + h
                    lo = blk_lo(i)
                    vix = (b * 16 + i) * 256 + cb * P
                    nc.tensor.matmul(pv_ps[:, lo:], v_all[:, vix:vix + P],
                                     ex[:, h, lo:],
                                     start=(i == 0), stop=(i == nkb - 1))
                # rowsum: pre-sum 4 exp blocks, one ones-matmul per quad
                quad_buf.append(ex)
                if len(quad_buf) == 2:
                    e0, e1 = quad_buf
                    quad_buf.clear()
                    qi = ii // 2
                    ea = tmp_pool.tile([P, QBLK], bf, tag="esA",
                                       bufs=2, name=f"esA_{b}{cb}{qj}_{qi}")
                    nc.vector.tensor_add(ea[:], e0[:, 0, :], e0[:, 1, :])
                    eb = tmp_pool.tile([P, QBLK], bf, tag="esB",
                                       bufs=2, name=f"esB_{b}{cb}{qj}_{qi}")
                    nc.gpsimd.tensor_add(eb[:], e1[:, 0, :], e1[:, 1, :])
                    es = tmp_pool.tile([P, QBLK], bf, tag="esC",
                                       bufs=2, name=f"esC_{b}{cb}{qj}_{qi}")
                    nc.vector.tensor_add(es[:], ea[:], eb[:])
                    nc.tensor.matmul(rs_ps[:], ones_sb[:], es[:],
                                     start=(qi == 0), stop=(qi == nquads - 1))

            pending = []

            def pair_step(ii, filler):
                sc_ps = psum.tile([P, 2, QBLK], f32, tag="sc",
                                  name=f"sc_{b}{cb}{qj}_{ii}")
                ex = exp_pool.tile([P, 2, QBLK], bf, tag="exp",
                                   name=f"ex_{b}{cb}{qj}_{ii}")
                los = []
                for h in range(2):
                    i = 2 * ii + h
                    lo = blk_lo(i)
                    los.append(lo)
                    ks = cb * T + b * S + i * P
                    nc.tensor.matmul(sc_ps[:, h, lo:], k_all[:, ks:ks + P],
                                     q_all[:, qs + lo:qs + QBLK],
                                     start=True, stop=True)
                # masked columns of ex must be exact zero for the rowsum
                for h in range(2):
                    if los[h] > 0:
                        nc.gpsimd.memset(ex[:, h, 0:los[h]], 0.0)
                if los[0] == 0 and los[1] == 0:
                    nc.scalar.activation(ex[:], sc_ps[:], Act.Exp,
                                         scale=SCALE)
                else:
                    for h in range(2):
                        nc.scalar.activation(ex[:, h, los[h]:],
                                             sc_ps[:, h, los[h]:],
                                             Act.Exp, scale=SCALE)
                # per-block causal triangle on the diagonal 128 columns
                for h in range(2):
                    i = 2 * ii + h
                    if i >= 4 * qj:
                        lo = los[h]
                        nc.gpsimd.tensor_mul(ex[:, h, lo:lo + P],
                                             ex[:, h, lo:lo + P], tri_sb[:])
                # independent PE work lands here, between the exp issue and
                # the rs/pv matmuls two pairs back (PE executes in order)
                filler()
                pending.append((ii, ex))
                if len(pending) == 3:
                    consume(*pending.pop(0))

            def finish(filler):
                consume(*pending.pop(0))
                filler()
                while pending:
                    consume(*pending.pop(0))
                rec = rec_pool.tile([P, QBLK], f32, tag="rec",
                                    name=f"rec_{b}{cb}{qj}")
                nc.vector.reciprocal_approx_fast(rec[:], rs_ps[:])
                nc.vector.tensor_mul(at_all[:, qs:qs + QBLK], pv_ps[:], rec[:])

            steps = [(lambda f, ii=ii: pair_step(ii, f))
                     for ii in range(nkb // 2)]
            steps.append(finish)
            return steps

        def out_units(psum, b, qj, last=False):
            # output projection for the 4 token chunks of (b, qj), split
            # into per-(token, dcol) units so they can fill PE bubbles
            # inside the next attention group's exp-chain.
            units = []

            def unit(tx, dc, orow_box):
                tt = (b * S + qj * QBLK) // P + tx
                if dc == 0:
                    orow_box.append(orow_pool.tile([P, D], bf, tag="orow",
                                                   name=f"orow_{tt}"))
                orow = orow_box[0]
                ps = psum.tile([P, 512], f32, tag="out",
                               name=f"out_{tt}_{dc}")
                for cb in range(2):
                    nc.tensor.matmul(
                        ps[:],
                        at_all[:, cb * T + tt * P:cb * T + (tt + 1) * P],
                        wo_sb[:, cb * D + dc * 512:cb * D + (dc + 1) * 512],
                        start=(cb == 0), stop=(cb == 1),
                    )
                # alternate eviction engine between ACT and DVE
                dst = orow[:, dc * 512:(dc + 1) * 512]
                if dc % 2 == 0:
                    nc.scalar.activation(dst, ps[:], Act.Copy)
                else:
                    nc.vector.tensor_copy(dst, ps[:])
                if dc == D // 512 - 1:
                    # drain the final group's rows on the otherwise-idle
                    # gpsimd ring so the tail DMA halves
                    eng = nc.gpsimd if last and tx >= 2 else nc.sync
                    eng.dma_start(out[tt * P:(tt + 1) * P, :], orow[:])

            for tx in range(QBLK // P):
                box = []
                for dc in range(D // 512):
                    units.append(lambda tx=tx, dc=dc, box=box: unit(tx, dc, box))
            return units

        def chunk7_units(psum):
            # QKV + RoPE for the last token chunk, emitted as PE-filler
            # units inside the b=0 attention section (whose groups don't
            # depend on it). PSUM accumulators borrow the "out" tag slots.
            t0 = (NCH - 1) * TCH
            xt = [xt_pool.tile([P, GA, TCH], bf, tag=f"xt{g}",
                               name=f"xt{g}_7") for g in range(NG)]
            for g in range(NG):
                nc.sync.dma_start(xt[g][:], xT4[:, g, :, t0:t0 + TCH])
            cosc = cs_pool.tile([P, 2, TCH], bf, tag="cos", name="cosc_7")
            nc.gpsimd.dma_start(cosc[:], cosT3[:, :, t0:t0 + TCH])
            sinc = cs_pool.tile([P, 2, TCH], bf, tag="sin", name="sinc_7")
            nc.gpsimd.dma_start(sinc[:], sinT3[:, :, t0:t0 + TCH])
            qraw = raw_pool.tile([P, 2, TCH], bf, tag="qraw", name="qraw_7")
            kraw = raw_pool.tile([P, 2, TCH], bf, tag="kraw", name="kraw_7")

            def qk_unit(wt, rawt, cb, which):
                ps = psum.tile([P, TCH], f32, tag="out",
                               name=f"c7qk_{which}{cb}")
                for a in range(NB):
                    nc.tensor.matmul(
                        ps[:], wt[a // GA][:, a % GA, cb * P:cb * P + P],
                        xt[a // GA][:, a % GA, :],
                        start=(a == 0), stop=(a == NB - 1))
                nc.scalar.activation(rawt[:, cb, :], ps[:], Act.Copy)

            def v_unit(tt):
                ps = psum.tile([P, 256], f32, tag="out", name=f"c7v_{tt}")
                for a in range(NB):
                    nc.tensor.matmul(
                        ps[:], xt[a // GA][:, a % GA, tt * P:(tt + 1) * P],
                        wv_sb[a // GA][:, a % GA, :],
                        start=(a == 0), stop=(a == NB - 1))
                cidx = ((NCH - 1) * (TCH // P) + tt) * 256
                nc.scalar.activation(v_all[:, cidx:cidx + 256], ps[:],
                                     Act.Copy)

            def rope_unit(rawt, dst):
                for cb in range(2):
                    tm = tmp_pool.tile([P, TCH], bf, tag="ropetmp")
                    nc.vector.tensor_mul(tm[:], rawt[:, 1 - cb, :],
                                         sinc[:, cb, :])
                    tm2 = tmp_pool.tile([P, TCH], bf, tag="ropetmp2")
                    nc.vector.tensor_mul(tm2[:], rawt[:, cb, :],
                                         cosc[:, cb, :])
                    nc.vector.tensor_add(
                        dst[:, cb * T + t0:cb * T + t0 + TCH], tm[:], tm2[:])

            return [
                lambda: qk_unit(wq_sb, qraw, 0, "q"),
                lambda: qk_unit(wq_sb, qraw, 1, "q"),
                lambda: qk_unit(wk_sb, kraw, 0, "k"),
                lambda: qk_unit(wk_sb, kraw, 1, "k"),
                lambda: v_unit(0), lambda: v_unit(1),
                lambda: v_unit(2), lambda: v_unit(3),
                lambda: rope_unit(qraw, q_all),
                lambda: rope_unit(kraw, k_all),
            ]

        with tc.tile_pool(name="psum2", bufs=2, space="PSUM") as psum:
            groups = [(b, qj) for b in range(B) for qj in range(NQ)]
            c7 = chunk7_units(psum)
            for gi, (b, qj) in enumerate(groups):
                steps = attn_group(psum, b, 0, qj) + attn_group(psum, b, 1, qj)
                outs = out_units(psum, *groups[gi - 1]) if gi >= 1 else []
                if gi < 4:  # spread chunk-7 work over the b=0 groups
                    outs = outs + c7[gi * 3:min((gi + 1) * 3, len(c7))]
                k = 0
                for si, st in enumerate(steps):
                    tgt = (si + 1) * len(outs) // len(steps)

                    def filler(tgt=tgt, outs=outs):
                        nonlocal k
                        while k < tgt:
                            outs[k]()
                            k += 1
                    st(filler)
            for u in out_units(psum, *groups[-1], last=True):
                u()

    nc.compile()
    return nc


def _host_prep(x, cos, sin, Wq, Wk, Wv, Wo):
    """Build per-core input maps (numpy, bf16 on-device dtypes)."""
    def pblock(arr, nblk):
        # [nblk*128, F] -> [128, nblk*F] with col = a*F + f
        nb, f = nblk, arr.shape[1]
        return np.ascontiguousarray(
            arr.reshape(nb, P, f).transpose(1, 0, 2).reshape(P, nb * f))

    x2 = np.asarray(x, np.float32).reshape(T, D)
    xT_r = pblock(np.ascontiguousarray(x2.T), NB).astype(BF16)  # [128, 16*4096]

    cosn = np.asarray(cos, np.float32)
    sinn = np.asarray(sin, np.float32)
    Wqn = np.asarray(Wq, np.float32)
    Wkn = np.asarray(Wk, np.float32)
    Wvn = np.asarray(Wv, np.float32)
    Won = np.asarray(Wo, np.float32)

    # intra-block causal triangle: tri[kj, qq] = (qq >= kj)
    kj = np.arange(P)[:, None]
    qq = np.arange(P)[None, :]
    tri = (qq >= kj).astype(np.float32)

    common = {
        "xT": xT_r,
        "tri": tri.astype(BF16),
        "ones": np.ones((P, P), BF16),
    }

    in_maps = []
    for m in range(N_CORES):
        cols = np.r_[128 * m:128 * m + 128, 1024 + 128 * m:1024 + 128 * m + 128]
        wq_s = pblock(Wqn[:, cols], NB).astype(BF16)
        wk_s = pblock(Wkn[:, cols], NB).astype(BF16)
        wv_s = pblock(Wvn[:, cols], NB).astype(BF16)
        wo_s = pblock(Won[cols, :], 2).astype(BF16)

        ct = np.tile(cosn[:, cols].T, (1, B))          # [256, 4096]
        st = np.tile(sinn[:, cols].T, (1, B)).copy()
        st[:128] *= -1.0                               # sign-fold block0
        cos_s = pblock(ct, 2).astype(BF16)
        sin_s = pblock(st, 2).astype(BF16)

        in_maps.append(dict(common, wq=wq_s, wk=wk_s, wv=wv_s, wo=wo_s,
                            cosT=cos_s, sinT=sin_s))
    return in_maps


def _numpy_fallback(x, cos, sin, Wq, bq, Wk, bk, Wv, bv, Wo, bo):
    """Exact fp32 reference path (only used when bq/bk are nonzero,
    which the spec's zero-filled biases never trigger)."""
    b, s, d = x.shape
    x2 = np.asarray(x, np.float32)
    q = x2 @ Wq + bq
    k = x2 @ Wk + bk
    v = x2 @ Wv + bv

    def rope(t):
        neg = np.concatenate([-t[..., d // 2:], t[..., :d // 2]], axis=-1)
        return t * cos[:s] + neg * sin[:s]

    q = rope(q).reshape(b, s, H, HD)
    k = rope(k).reshape(b, s, H, HD)
    v = v.reshape(b, s, H, HD)
    sc = np.einsum('bqhd,bkhd->bhqk', q, k) / np.sqrt(HD)
    mask = np.tril(np.ones((s, s), bool))
    sc = np.where(mask, sc, -np.inf)
    sc -= sc.max(-1, keepdims=True)
    p = np.exp(sc)
    p /= p.sum(-1, keepdims=True)
    at = np.einsum('bhqk,bkhd->bqhd', p, v).reshape(b, s, d)
    return at @ Wo + bo


def kernel(x, cos, sin, Wq, bq, Wk, bk, Wv, bv, Wo, bo):
    global LAST_RESULTS
    from concourse.bass_utils import run_bass_kernel_spmd

    if np.any(np.asarray(bq)) or np.any(np.asarray(bk)):
        return _numpy_fallback(x, cos, sin,
                               np.asarray(Wq, np.float32), np.asarray(bq, np.float32),
                               np.asarray(Wk, np.float32), np.asarray(bk, np.float32),
                               np.asarray(Wv, np.float32), np.asarray(bv, np.float32),
                               np.asarray(Wo, np.float32), np.asarray(bo, np.float32))

    if "nc" not in _CACHE:
        _CACHE["nc"] = _build_program()
    nc = _CACHE["nc"]

    in_maps = _host_prep(x, cos, sin, Wq, Wk, Wv, Wo)
    res = run_bass_kernel_spmd(nc, in_maps, core_ids=list(range(N_CORES)))
    LAST_RESULTS = res

    acc = np.zeros((T, D), np.float32)
    for r in res.results:
        acc += r["out"].astype(np.float32)
    # v-bias and output bias: attn rows sum to 1, so bv contributes bv @ Wo.
    acc += (np.asarray(bv, np.float32) @ np.asarray(Wo, np.float32)
            + np.asarray(bo, np.float32))[None, :]
    return acc.reshape(B, S, D)


# revision 7
# speedup vs baseline: 1.0202x; 1.0202x over previous
"""Trainium2 Bass kernel for nn_Attention_14877766713476.

Causal multi-head attention with full-hidden RoPE:
  q,k,v = x@W{q,k,v} + b;  q,k = rope(q,k);  heads=16, hd=128;
  causal softmax attention;  out = attn@Wo + bo.

Sharding: tensor-parallel over heads across 8 cores. RoPE pairs hidden
column c with c +/- 1024, i.e. head h with head h+8 -- so core m owns
heads {m, m+8} and RoPE stays core-local. Each core computes its two
heads end-to-end and a partial output projection (rows of Wo); the host
sums the 8 partials.

All matmuls in bf16 with fp32 PSUM accumulation. Host pre-transposes
x -> xT (contraction dim on partitions) and pre-slices/casts weights,
so the device does zero transposes.

v2 changes over the baseline:
  - biases dropped on device (spec fills them with zeros; host numpy
    fallback covers the general case), raw q/k evicted via ScalarE.
  - diagonal score blocks compute only their unmasked column range
    (scores, exp and PV all shrink); the per-block causal triangle is
    a single [128,128] mask on GpSimd; masked columns of the exp tile
    are zeroed by small GpSimd memsets (for the rowsum).
  - attention inner loop is pipelined 2 pairs deep so PE never waits
    on ScalarE's exp.
  - exp-rowsum presums split between DVE and GpSimd.
  - prologue DMA order interleaves wq/x groups on the sync ring while
    wk/wv/cos/sin stream on the gpsimd ring.

Layouts (per core, host-prepared, all bf16 unless noted):
  xT    [128, 16*4096]  col = a*4096 + t   (d = a*128 + p, t = b*2048+s)
  wq/wk/wv [128, 16*256] col = a*256 + c   (d = a*128 + p, c in 0..255)
  wo    [128, 2*2048]   col = cb*2048 + dcol  (c = cb*128 + p)
  cosT/sinT [128, 2*4096] col = cb*4096 + t   (c = cb*128 + p; sinT block0
                           negated so rope_b = q_b*cos_b + q_{1-b}*sinT_b)
  tri   [128, 128]      tri[kj, qq] = (qq >= kj)  (intra-block causal)
  ones  [128, 128]      all ones (rowsum matmul stationary)
Output per core: out [4096, 2048] bf16 partial (this core's two heads
through Wo rows); host sums partials in fp32 and adds bv@Wo + bo.
"""

import math
from contextlib import ExitStack

import numpy as np
import ml_dtypes

N_CORES = 8
B, S, D, H = 2, 2048, 2048, 16
HD = D // H          # 128
T = B * S            # 4096
P = 128
NB = D // P          # 16 contraction blocks
NG = 4               # a-groups (DMA split granularity)
GA = NB // NG        # 4 a-blocks per group
TCH = 512            # token chunk (QKV phase free dim)
NCH = T // TCH       # 8
QBLK = 512           # query block (attention phase free dim)
NQ = S // QBLK       # 4 query blocks per (batch, head)
SCALE = 1.0 / math.sqrt(HD)

BF16 = ml_dtypes.bfloat16

_CACHE = {}
LAST_RESULTS = None


def _build_program():
    import concourse.tile as tile
    from concourse import bacc, mybir

    bf = mybir.dt.bfloat16
    f32 = mybir.dt.float32
    Act = mybir.ActivationFunctionType

    nc = bacc.Bacc("TRN2", target_bir_lowering=False, debug=False,
                   num_devices=N_CORES)

    xT = nc.dram_tensor("xT", [P, NB * T], bf, kind="ExternalInput").ap()
    wq = nc.dram_tensor("wq", [P, NB * 256], bf, kind="ExternalInput").ap()
    wk = nc.dram_tensor("wk", [P, NB * 256], bf, kind="ExternalInput").ap()
    wv = nc.dram_tensor("wv", [P, NB * 256], bf, kind="ExternalInput").ap()
    wo = nc.dram_tensor("wo", [P, 2 * D], bf, kind="ExternalInput").ap()
    cosT = nc.dram_tensor("cosT", [P, 2 * T], bf, kind="ExternalInput").ap()
    sinT = nc.dram_tensor("sinT", [P, 2 * T], bf, kind="ExternalInput").ap()
    tri = nc.dram_tensor("tri", [P, P], bf, kind="ExternalInput").ap()
    ones = nc.dram_tensor("ones", [P, P], bf, kind="ExternalInput").ap()
    out = nc.dram_tensor("out", [T, D], bf, kind="ExternalOutput").ap()

    xT4 = xT.rearrange("p (g a t) -> p g a t", g=NG, a=GA)
    wq4 = wq.rearrange("p (g a c) -> p g a c", g=NG, a=GA)
    wk4 = wk.rearrange("p (g a c) -> p g a c", g=NG, a=GA)
    wv4 = wv.rearrange("p (g a c) -> p g a c", g=NG, a=GA)
    cosT3 = cosT.rearrange("p (c t) -> p c t", c=2)
    sinT3 = sinT.rearrange("p (c t) -> p c t", c=2)

    with tile.TileContext(nc) as tc, ExitStack() as ctx:
        const = ctx.enter_context(tc.tile_pool(name="const", bufs=1))
        persist = ctx.enter_context(tc.tile_pool(name="persist", bufs=1))
        xt_pool = ctx.enter_context(tc.tile_pool(name="xt", bufs=2))
        cs_pool = ctx.enter_context(tc.tile_pool(name="cs", bufs=2))
        raw_pool = ctx.enter_context(tc.tile_pool(name="raw", bufs=2))
        tmp_pool = ctx.enter_context(tc.tile_pool(name="tmp", bufs=4))
        exp_pool = ctx.enter_context(tc.tile_pool(name="exp", bufs=5))
        rec_pool = ctx.enter_context(tc.tile_pool(name="rec", bufs=2))
        orow_pool = ctx.enter_context(tc.tile_pool(name="orow", bufs=2))

        # Weight/x prologue: interleave (wq_g, xt_g) pairs on the sync
        # ring so the q accumulation chain never outruns its weights;
        # wk/wv/cos/sin stream in parallel on the gpsimd ring, ordered
        # by first use (k matmuls, v matmuls, then RoPE). wo is loaded
        # later, on the sync ring behind the chunk-3 x stream, so it
        # doesn't steal prologue bandwidth.
        wq_sb = [const.tile([P, GA, 256], bf, tag=f"wq{g}", name=f"wq_sb{g}")
                 for g in range(NG)]
        wk_sb = [const.tile([P, GA, 256], bf, tag=f"wk{g}", name=f"wk_sb{g}")
                 for g in range(NG)]
        wv_sb = [const.tile([P, GA, 256], bf, tag=f"wv{g}", name=f"wv_sb{g}")
                 for g in range(NG)]
        xt0 = [xt_pool.tile([P, GA, TCH], bf, tag=f"xt{g}", name=f"xt{g}_0")
               for g in range(NG)]
        nc.sync.dma_start(wq_sb[0][:], wq4[:, 0])
        nc.sync.dma_start(xt0[0][:, 0:2, :], xT4[:, 0, 0:2, 0:TCH])
        nc.sync.dma_start(xt0[0][:, 2:4, :], xT4[:, 0, 2:4, 0:TCH])
        for g in range(1, NG):
            nc.sync.dma_start(wq_sb[g][:], wq4[:, g])
            nc.sync.dma_start(xt0[g][:], xT4[:, g, :, 0:TCH])
        nc.gpsimd.dma_start(wk_sb[0][:], wk4[:, 0])
        nc.gpsimd.dma_start(wv_sb[0][:], wv4[:, 0])
        nc.gpsimd.dma_start(wk_sb[1][:], wk4[:, 1])
        nc.gpsimd.dma_start(wv_sb[1][:], wv4[:, 1])

        cos0 = cs_pool.tile([P, 2, TCH], bf, tag="cos", name="cosc_0")
        nc.gpsimd.dma_start(cos0[:], cosT3[:, :, 0:TCH])
        sin0 = cs_pool.tile([P, 2, TCH], bf, tag="sin", name="sinc_0")
        nc.gpsimd.dma_start(sin0[:], sinT3[:, :, 0:TCH])

        for g in (2, 3):
            nc.gpsimd.dma_start(wk_sb[g][:], wk4[:, g])
            nc.gpsimd.dma_start(wv_sb[g][:], wv4[:, g])

        wo_sb = const.tile([P, 2 * D], bf, tag="wo")
        tri_sb = const.tile([P, P], bf, tag="tri")
        ones_sb = const.tile([P, P], bf, tag="ones")
        nc.gpsimd.dma_start(tri_sb[:], tri[:])
        nc.gpsimd.dma_start(ones_sb[:], ones[:])

        # persistent activations
        q_all = persist.tile([P, 2 * T], bf, tag="q_all")      # roped qT
        k_all = persist.tile([P, 2 * T], bf, tag="k_all")      # roped kT
        v_all = persist.tile([P, 32 * 256], bf, tag="v_all")   # v natural
        at_all = persist.tile([P, 2 * T], bf, tag="at_all")    # attnT

        # ---------------- Phase 1: QKV projections + RoPE ----------------
        with tc.tile_pool(name="psum1", bufs=4, space="PSUM") as psum:
            for tcix in range(NCH - 1):
                t0 = tcix * TCH
                if tcix == 0:
                    xt = xt0
                    cosc, sinc = cos0, sin0
                else:
                    xt = [xt_pool.tile([P, GA, TCH], bf, tag=f"xt{g}",
                                       name=f"xt{g}_{tcix}")
                          for g in range(NG)]
                    for g in range(NG):
                        nc.sync.dma_start(xt[g][:], xT4[:, g, :, t0:t0 + TCH])
                    cosc = cs_pool.tile([P, 2, TCH], bf, tag="cos")
                    nc.sync.dma_start(cosc[:], cosT3[:, :, t0:t0 + TCH])
                    sinc = cs_pool.tile([P, 2, TCH], bf, tag="sin")
                    nc.sync.dma_start(sinc[:], sinT3[:, :, t0:t0 + TCH])
                if tcix == 3:
                    # wo behind the chunk-3 x stream on the sync queue:
                    # arrives long before the first output projection.
                    nc.sync.dma_start(wo_sb[:], wo[:])

                qraw = raw_pool.tile([P, 2, TCH], bf, tag="qraw")
                kraw = raw_pool.tile([P, 2, TCH], bf, tag="kraw")
                if tcix == 0:
                    # group-interleaved order: each (wq_g, xt_g, wk_g, wv_g)
                    # DMA group unlocks its matmuls immediately, so PE
                    # starts after ~1 group of traffic instead of 3 MB.
                    qk_ps = [psum.tile([P, TCH], f32, tag="qk",
                                       name=f"c0qk{j}") for j in range(4)]
                    v_ps = [psum.tile([P, 256], f32, tag="v",
                                      name=f"c0v{tt}") for tt in range(4)]
                    for g in range(NG):
                        for j, (wt, cb) in enumerate(
                                ((wq_sb, 0), (wq_sb, 1),
                                 (wk_sb, 0), (wk_sb, 1))):
                            for al in range(GA):
                                a = g * GA + al
                                nc.tensor.matmul(
                                    qk_ps[j][:],
                                    wt[g][:, al, cb * P:cb * P + P],
                                    xt[g][:, al, :],
                                    start=(a == 0), stop=(a == NB - 1),
                                )
                        for tt in range(TCH // P):
                            for al in range(GA):
                                a = g * GA + al
                                nc.tensor.matmul(
                                    v_ps[tt][:],
                                    xt[g][:, al, tt * P:(tt + 1) * P],
                                    wv_sb[g][:, al, :],
                                    start=(a == 0), stop=(a == NB - 1),
                                )
                    for j, (rawt, cb) in enumerate(
                            ((qraw, 0), (qraw, 1), (kraw, 0), (kraw, 1))):
                        nc.scalar.activation(rawt[:, cb, :], qk_ps[j][:],
                                             Act.Copy)
                    for tt in range(TCH // P):
                        nc.scalar.activation(v_all[:, tt * 256:(tt + 1) * 256],
                                             v_ps[tt][:], Act.Copy)
                else:
                    for (wt, rawt) in ((wq_sb, qraw), (wk_sb, kraw)):
                        for cb in range(2):
                            ps = psum.tile([P, TCH], f32, tag="qk")
                            for a in range(NB):
                                nc.tensor.matmul(
                                    ps[:],
                                    wt[a // GA][:, a % GA,
                                                cb * P:cb * P + P],
                                    xt[a // GA][:, a % GA, :],
                                    start=(a == 0), stop=(a == NB - 1),
                                )
                            nc.scalar.activation(rawt[:, cb, :], ps[:],
                                                 Act.Copy)
                    # v: x-stationary, natural layout
                    for tt in range(TCH // P):
                        ps = psum.tile([P, 256], f32, tag="v")
                        for a in range(NB):
                            nc.tensor.matmul(
                                ps[:],
                                xt[a // GA][:, a % GA, tt * P:(tt + 1) * P],
                                wv_sb[a // GA][:, a % GA, :],
                                start=(a == 0), stop=(a == NB - 1),
                            )
                        cidx = (tcix * (TCH // P) + tt) * 256
                        nc.scalar.activation(v_all[:, cidx:cidx + 256], ps[:],
                                             Act.Copy)

                # RoPE: rope_b = raw_b*cos_b + raw_{1-b}*sinT_b (sign-folded)
                for (rawt, dst) in ((qraw, q_all), (kraw, k_all)):
                    for cb in range(2):
                        tm = tmp_pool.tile([P, TCH], bf, tag="ropetmp")
                        nc.vector.tensor_mul(tm[:], rawt[:, 1 - cb, :],
                                             sinc[:, cb, :])
                        tm2 = tmp_pool.tile([P, TCH], bf, tag="ropetmp2")
                        nc.vector.tensor_mul(tm2[:], rawt[:, cb, :],
                                             cosc[:, cb, :])
                        nc.vector.tensor_add(
                            dst[:, cb * T + t0:cb * T + t0 + TCH],
                            tm[:], tm2[:])

        # ------- Phase 2+3: causal attention + output projection -------
        # scoresT blocks [kj=128, q=512]; diagonal blocks shrink to their
        # unmasked column range; exp on ScalarE; per-block causal triangle
        # via a [128,128] GpSimd mask; rowsum via ones-matmul on quad
        # presums (DVE+GpSimd); PV consumes expT directly. The inner loop
        # is software-pipelined two pairs deep (rs/pv trail sc/exp by two
        # pairs) so PE never head-of-line blocks on ScalarE's exp. The
        # output projection for a (b, qj) token group is interleaved one
        # group late, once its at_all slices are long since written.
        def attn_group(psum, b, cb, qj):
            qs = cb * T + b * S + qj * QBLK
            nkb = 4 * qj + 4  # key blocks 0..nkb-1
            npair = nkb // 2
            nquads = nkb // 4
            pv_ps = psum.tile([P, QBLK], f32, tag="pv", bufs=1,
                              name=f"pv_{b}{cb}{qj}")
            rs_ps = psum.tile([P, QBLK], f32, tag="rs", bufs=1,
                              name=f"rs_{b}{cb}{qj}")
            quad_buf = []
            counts = {"pv": 0, "rs": 0}

            def blk_lo(i):
                # first unmasked column of key block i within this q block
                dd = i - 4 * qj
                return 128 * dd if dd > 0 else 0

            def consume(ii, ex):
                for h in range(2):
                    i = 2 * ii + h
                    lo = blk_lo(i)
                    # first matmul of the accumulation must cover the full
                    # bank (start zeroes it); masked ex columns are zero.
                    if counts["pv"] == 0:
                        lo = 0
                    vix = (b * 16 + i) * 256 + cb * P
                    nc.tensor.matmul(pv_ps[:, lo:], v_all[:, vix:vix + P],
                                     ex[:, h, lo:],
                                     start=(counts["pv"] == 0),
                                     stop=(counts["pv"] == nkb - 1))
                    counts["pv"] += 1
                # rowsum: pre-sum 4 exp blocks, one ones-matmul per quad
                quad_buf.append(ex)
                if len(quad_buf) == 2:
                    e0, e1 = quad_buf
                    quad_buf.clear()
                    qi = counts["rs"]
                    ea = tmp_pool.tile([P, QBLK], bf, tag="esA",
                                       bufs=2, name=f"esA_{b}{cb}{qj}_{qi}")
                    nc.vector.tensor_add(ea[:], e0[:, 0, :], e0[:, 1, :])
                    eb = tmp_pool.tile([P, QBLK], bf, tag="esB",
                                       bufs=2, name=f"esB_{b}{cb}{qj}_{qi}")
                    # last quad stays on DVE so the group tail never waits
                    # on the slower gpsimd engine
                    eb_eng = nc.vector if qi == nquads - 1 else nc.gpsimd
                    eb_eng.tensor_add(eb[:], e1[:, 0, :], e1[:, 1, :])
                    es = tmp_pool.tile([P, QBLK], bf, tag="esC",
                                       bufs=2, name=f"esC_{b}{cb}{qj}_{qi}")
                    nc.vector.tensor_add(es[:], ea[:], eb[:])
                    nc.tensor.matmul(rs_ps[:], ones_sb[:], es[:],
                                     start=(qi == 0), stop=(qi == nquads - 1))
                    counts["rs"] += 1

            pending = []

            def pair_step(ii, filler):
                sc_ps = psum.tile([P, 2, QBLK], f32, tag="sc",
                                  name=f"sc_{b}{cb}{qj}_{ii}")
                ex = exp_pool.tile([P, 2, QBLK], bf, tag="exp",
                                   name=f"ex_{b}{cb}{qj}_{ii}")
                los = []
                for h in range(2):
                    i = 2 * ii + h
                    lo = blk_lo(i)
                    los.append(lo)
                    ks = cb * T + b * S + i * P
                    nc.tensor.matmul(sc_ps[:, h, lo:], k_all[:, ks:ks + P],
                                     q_all[:, qs + lo:qs + QBLK],
                                     start=True, stop=True)
                # masked columns of ex must be exact zero for the rowsum
                for h in range(2):
                    if los[h] > 0:
                        nc.gpsimd.memset(ex[:, h, 0:los[h]], 0.0)
                if los[0] == 0 and los[1] == 0:
                    nc.scalar.activation(ex[:], sc_ps[:], Act.Exp,
                                         scale=SCALE)
                else:
                    for h in range(2):
                        nc.scalar.activation(ex[:, h, los[h]:],
                                             sc_ps[:, h, los[h]:],
                                             Act.Exp, scale=SCALE)
                # per-block causal triangle on the diagonal 128 columns
                for h in range(2):
                    i = 2 * ii + h
                    if i >= 4 * qj:
                        lo = los[h]
                        nc.vector.tensor_mul(ex[:, h, lo:lo + P],
                                             ex[:, h, lo:lo + P], tri_sb[:])
                # independent PE work lands here, between the exp issue and
                # the rs/pv matmuls two pairs back (PE executes in order)
                filler()
                pending.append((ii, ex))
                if len(pending) == 3:
                    consume(*pending.pop(0))

            def finish(filler):
                consume(*pending.pop(0))
                filler()
                while pending:
                    consume(*pending.pop(0))
                rec = rec_pool.tile([P, QBLK], f32, tag="rec",
                                    name=f"rec_{b}{cb}{qj}")
                nc.vector.reciprocal_approx_fast(rec[:], rs_ps[:])
                nc.vector.tensor_mul(at_all[:, qs:qs + QBLK], pv_ps[:], rec[:])

            # diagonal pairs first: their exp -> triangle-mask chain then
            # overlaps the dense pairs' matmuls instead of the group tail.
            steps = [(lambda f, ii=ii: pair_step(ii, f))
                     for ii in range(npair - 1, -1, -1)]
            steps.append(finish)
            return steps

        def out_units(psum, b, qj, last=False):
            # output projection for the 4 token chunks of (b, qj), split
            # into per-(token, dcol) units so they can fill PE bubbles
            # inside the next attention group's exp-chain.
            units = []

            def unit(tx, dc, orow_box):
                tt = (b * S + qj * QBLK) // P + tx
                if dc == 0:
                    orow_box.append(orow_pool.tile([P, D], bf, tag="orow",
                                                   name=f"orow_{tt}"))
                orow = orow_box[0]
                ps = psum.tile([P, 512], f32, tag="out",
                               name=f"out_{tt}_{dc}")
                for cb in range(2):
                    nc.tensor.matmul(
                        ps[:],
                        at_all[:, cb * T + tt * P:cb * T + (tt + 1) * P],
                        wo_sb[:, cb * D + dc * 512:cb * D + (dc + 1) * 512],
                        start=(cb == 0), stop=(cb == 1),
                    )
                # alternate eviction engine between ACT and DVE
                dst = orow[:, dc * 512:(dc + 1) * 512]
                if dc % 2 == 0:
                    nc.scalar.activation(dst, ps[:], Act.Copy)
                else:
                    nc.vector.tensor_copy(dst, ps[:])
                if dc == D // 512 - 1:
                    # drain the final group's rows on the otherwise-idle
                    # gpsimd ring so the tail DMA halves
                    eng = nc.gpsimd if last and tx >= 2 else nc.sync
                    eng.dma_start(out[tt * P:(tt + 1) * P, :], orow[:])

            for tx in range(QBLK // P):
                box = []
                for dc in range(D // 512):
                    units.append(lambda tx=tx, dc=dc, box=box: unit(tx, dc, box))
            return units

        def chunk7_units(psum):
            # QKV + RoPE for the last token chunk, emitted as PE-filler
            # units inside the b=0 attention section (whose groups don't
            # depend on it). PSUM accumulators borrow the "out" tag slots.
            t0 = (NCH - 1) * TCH
            xt = [xt_pool.tile([P, GA, TCH], bf, tag=f"xt{g}",
                               name=f"xt{g}_7") for g in range(NG)]
            for g in range(NG):
                nc.sync.dma_start(xt[g][:], xT4[:, g, :, t0:t0 + TCH])
            cosc = cs_pool.tile([P, 2, TCH], bf, tag="cos", name="cosc_7")
            nc.gpsimd.dma_start(cosc[:], cosT3[:, :, t0:t0 + TCH])
            sinc = cs_pool.tile([P, 2, TCH], bf, tag="sin", name="sinc_7")
            nc.gpsimd.dma_start(sinc[:], sinT3[:, :, t0:t0 + TCH])
            qraw = raw_pool.tile([P, 2, TCH], bf, tag="qraw", name="qraw_7")
            kraw = raw_pool.tile([P, 2, TCH], bf, tag="kraw", name="kraw_7")

            def qk_unit(wt, rawt, cb, which):
                ps = psum.tile([P, TCH], f32, tag="out",
                               name=f"c7qk_{which}{cb}")
                for a in range(NB):
                    nc.tensor.matmul(
                        ps[:], wt[a // GA][:, a % GA, cb * P:cb * P + P],
                        xt[a // GA][:, a % GA, :],
                        start=(a == 0), stop=(a == NB - 1))
                nc.scalar.activation(rawt[:, cb, :], ps[:], Act.Copy)

            def v_unit(tt):
                ps = psum.tile([P, 256], f32, tag="out", name=f"c7v_{tt}")
                for a in range(NB):
                    nc.tensor.matmul(
                        ps[:], xt[a // GA][:, a % GA, tt * P:(tt + 1) * P],
                        wv_sb[a // GA][:, a % GA, :],
                        start=(a == 0), stop=(a == NB - 1))
                cidx = ((NCH - 1) * (TCH // P) + tt) * 256
                nc.scalar.activation(v_all[:, cidx:cidx + 256], ps[:],
                                     Act.Copy)

            def rope_unit(rawt, dst):
                for cb in range(2):
                    tm = tmp_pool.tile([P, TCH], bf, tag="ropetmp")
                    nc.vector.tensor_mul(tm[:], rawt[:, 1 - cb, :],
                                         sinc[:, cb, :])
                    tm2 = tmp_pool.tile([P, TCH], bf, tag="ropetmp2")
                    nc.vector.tensor_mul(tm2[:], rawt[:, cb, :],
                                         cosc[:, cb, :])
                    nc.vector.tensor_add(
                        dst[:, cb * T + t0:cb * T + t0 + TCH], tm[:], tm2[:])

            return [
                lambda: qk_unit(wq_sb, qraw, 0, "q"),
                lambda: qk_unit(wq_sb, qraw, 1, "q"),
                lambda: qk_unit(wk_sb, kraw, 0, "k"),
                lambda: qk_unit(wk_sb, kraw, 1, "k"),
                lambda: v_unit(0), lambda: v_unit(1),
                lambda: v_unit(2), lambda: v_unit(3),
                lambda: rope_unit(qraw, q_all),
                lambda: rope_unit(kraw, k_all),
            ]

        with tc.tile_pool(name="psum2", bufs=2, space="PSUM") as psum:
            groups = [(b, qj) for b in range(B) for qj in range(NQ)]
            c7 = chunk7_units(psum)
            for gi, (b, qj) in enumerate(groups):
                steps = attn_group(psum, b, 0, qj) + attn_group(psum, b, 1, qj)
                outs = out_units(psum, *groups[gi - 1]) if gi >= 1 else []
                if gi < 4:  # spread chunk-7 work over the b=0 groups
                    outs = outs + c7[gi * 3:min((gi + 1) * 3, len(c7))]
                k = 0
                for si, st in enumerate(steps):
                    tgt = (si + 1) * len(outs) // len(steps)

                    def filler(tgt=tgt, outs=outs):
                        nonlocal k
                        while k < tgt:
                            outs[k]()
                            k += 1
                    st(filler)
            for u in out_units(psum, *groups[-1], last=True):
                u()

    nc.compile()
    return nc


def _host_prep(x, cos, sin, Wq, Wk, Wv, Wo):
    """Build per-core input maps (numpy, bf16 on-device dtypes)."""
    def pblock(arr, nblk):
        # [nblk*128, F] -> [128, nblk*F] with col = a*F + f
        nb, f = nblk, arr.shape[1]
        return np.ascontiguousarray(
            arr.reshape(nb, P, f).transpose(1, 0, 2).reshape(P, nb * f))

    x2 = np.asarray(x, np.float32).reshape(T, D)
    xT_r = pblock(np.ascontiguousarray(x2.T), NB).astype(BF16)  # [128, 16*4096]

    cosn = np.asarray(cos, np.float32)
    sinn = np.asarray(sin, np.float32)
    Wqn = np.asarray(Wq, np.float32)
    Wkn = np.asarray(Wk, np.float32)
    Wvn = np.asarray(Wv, np.float32)
    Won = np.asarray(Wo, np.float32)

    # intra-block causal triangle: tri[kj, qq] = (qq >= kj)
    kj = np.arange(P)[:, None]
    qq = np.arange(P)[None, :]
    tri = (qq >= kj).astype(np.float32)

    common = {
        "xT": xT_r,
        "tri": tri.astype(BF16),
        "ones": np.ones((P, P), BF16),
    }

    in_maps = []
    for m in range(N_CORES):
        cols = np.r_[128 * m:128 * m + 128, 1024 + 128 * m:1024 + 128 * m + 128]
        wq_s = pblock(Wqn[:, cols], NB).astype(BF16)
        wk_s = pblock(Wkn[:, cols], NB).astype(BF16)
        wv_s = pblock(Wvn[:, cols], NB).astype(BF16)
        wo_s = pblock(Won[cols, :], 2).astype(BF16)

        ct = np.tile(cosn[:, cols].T, (1, B))          # [256, 4096]
        st = np.tile(sinn[:, cols].T, (1, B)).copy()
        st[:128] *= -1.0                               # sign-fold block0
        cos_s = pblock(ct, 2).astype(BF16)
        sin_s = pblock(st, 2).astype(BF16)

        in_maps.append(dict(common, wq=wq_s, wk=wk_s, wv=wv_s, wo=wo_s,
                            cosT=cos_s, sinT=sin_s))
    return in_maps


def _numpy_fallback(x, cos, sin, Wq, bq, Wk, bk, Wv, bv, Wo, bo):
    """Exact fp32 reference path (only used when bq/bk are nonzero,
    which the spec's zero-filled biases never trigger)."""
    b, s, d = x.shape
    x2 = np.asarray(x, np.float32)
    q = x2 @ Wq + bq
    k = x2 @ Wk + bk
    v = x2 @ Wv + bv

    def rope(t):
        neg = np.concatenate([-t[..., d // 2:], t[..., :d // 2]], axis=-1)
        return t * cos[:s] + neg * sin[:s]

    q = rope(q).reshape(b, s, H, HD)
    k = rope(k).reshape(b, s, H, HD)
    v = v.reshape(b, s, H, HD)
    sc = np.einsum('bqhd,bkhd->bhqk', q, k) / np.sqrt(HD)
    mask = np.tril(np.ones((s, s), bool))
    sc = np.where(mask, sc, -np.inf)
    sc -= sc.max(-1, keepdims=True)
    p = np.exp(sc)
    p /= p.sum(-1, keepdims=True)
    at = np.einsum('bhqk,bkhd->bqhd', p, v).reshape(b, s, d)
    return at @ Wo + bo


def kernel(x, cos, sin, Wq, bq, Wk, bk, Wv, bv, Wo, bo):
    global LAST_RESULTS
    from concourse.bass_utils import run_bass_kernel_spmd

    if np.any(np.asarray(bq)) or np.any(np.asarray(bk)):
        return _numpy_fallback(x, cos, sin,
                               np.asarray(Wq, np.float32), np.asarray(bq, np.float32),
                               np.asarray(Wk, np.float32), np.asarray(bk, np.float32),
                               np.asarray(Wv, np.float32), np.asarray(bv, np.float32),
                               np.asarray(Wo, np.float32), np.asarray(bo, np.float32))

    if "nc" not in _CACHE:
        _CACHE["nc"] = _build_program()
    nc = _CACHE["nc"]

    in_maps = _host_prep(x, cos, sin, Wq, Wk, Wv, Wo)
    res = run_bass_kernel_spmd(nc, in_maps, core_ids=list(range(N_CORES)))
    LAST_RESULTS = res

    acc = np.zeros((T, D), np.float32)
    for r in res.results:
        acc += r["out"].astype(np.float32)
    # v-bias and output bias: attn rows sum to 1, so bv contributes bv @ Wo.
    acc += (np.asarray(bv, np.float32) @ np.asarray(Wo, np.float32)
            + np.asarray(bo, np.float32))[None, :]
    return acc.reshape(B, S, D)


# revision 18
# speedup vs baseline: 1.0562x; 1.0352x over previous
"""Trainium2 Bass kernel for nn_Attention_14877766713476.

Causal multi-head attention with full-hidden RoPE:
  q,k,v = x@W{q,k,v} + b;  q,k = rope(q,k);  heads=16, hd=128;
  causal softmax attention;  out = attn@Wo + bo.

Sharding: tensor-parallel over heads across 8 cores. RoPE pairs hidden
column c with c +/- 1024, i.e. head h with head h+8 -- so core m owns
heads {m, m+8} and RoPE stays core-local. Each core computes its two
heads end-to-end and a partial output projection (rows of Wo); the host
sums the 8 partials.

All matmuls in bf16 with fp32 PSUM accumulation. Host pre-transposes
x -> xT (contraction dim on partitions) and pre-slices/casts weights,
so the device does zero transposes.

v2 changes over the baseline:
  - biases dropped on device (spec fills them with zeros; host numpy
    fallback covers the general case), raw q/k evicted via ScalarE.
  - diagonal score blocks compute only their unmasked column range
    (scores, exp and PV all shrink); the per-block causal triangle is
    a single [128,128] mask on GpSimd; masked columns of the exp tile
    are zeroed by small GpSimd memsets (for the rowsum).
  - attention inner loop is pipelined 2 pairs deep so PE never waits
    on ScalarE's exp.
  - exp-rowsum presums split between DVE and GpSimd.
  - prologue DMA order interleaves wq/x groups on the sync ring while
    wk/wv/cos/sin stream on the gpsimd ring.

Layouts (per core, host-prepared, all bf16 unless noted):
  xT    [128, 16*4096]  col = a*4096 + t   (d = a*128 + p, t = b*2048+s)
  wq/wk/wv [128, 16*256] col = a*256 + c   (d = a*128 + p, c in 0..255)
  wo    [128, 2*2048]   col = cb*2048 + dcol  (c = cb*128 + p)
  cosT/sinT [128, 2*4096] col = cb*4096 + t   (c = cb*128 + p; sinT block0
                           negated so rope_b = q_b*cos_b + q_{1-b}*sinT_b)
  tri   [128, 128]      tri[kj, qq] = (qq >= kj)  (intra-block causal)
  ones  [128, 128]      all ones (rowsum matmul stationary)
Output per core: out [4096, 2048] bf16 partial (this core's two heads
through Wo rows); host sums partials in fp32 and adds bv@Wo + bo.
"""

import math
from contextlib import ExitStack

import numpy as np
import ml_dtypes

N_CORES = 8
B, S, D, H = 2, 2048, 2048, 16
HD = D // H          # 128
T = B * S            # 4096
P = 128
NB = D // P          # 16 contraction blocks
NG = 4               # a-groups (DMA split granularity)
GA = NB // NG        # 4 a-blocks per group
TCH = 512            # token chunk (QKV phase free dim)
NCH = T // TCH       # 8
QBLK = 512           # query block (attention phase free dim)
NQ = S // QBLK       # 4 query blocks per (batch, head)
SCALE = 1.0 / math.sqrt(HD)

BF16 = ml_dtypes.bfloat16

_CACHE = {}
LAST_RESULTS = None


def _build_program():
    import concourse.tile as tile
    from concourse import bacc, mybir

    bf = mybir.dt.bfloat16
    f32 = mybir.dt.float32
    Act = mybir.ActivationFunctionType

    nc = bacc.Bacc("TRN2", target_bir_lowering=False, debug=False,
                   num_devices=N_CORES)

    # DRAM layouts are chunk-major so every DMA descriptor reads fully
    # contiguous per-partition rows (16 KB for x, 4 KB for cos/sin) --
    # fragmented rows cost one DMA packet per 1 KB segment and cap the
    # aggregate stream far below HBM bandwidth.
    xT = nc.dram_tensor("xT", [P, NCH * NB * TCH], bf,
                        kind="ExternalInput").ap()
    wq = nc.dram_tensor("wq", [P, NB * 256], bf, kind="ExternalInput").ap()
    wkv = nc.dram_tensor("wkv", [P, NG * 2 * GA * 256], bf,
                         kind="ExternalInput").ap()
    wo = nc.dram_tensor("wo", [P, 2 * D], bf, kind="ExternalInput").ap()
    cs = nc.dram_tensor("cs", [P, NCH * 4 * TCH], bf,
                        kind="ExternalInput").ap()
    tri = nc.dram_tensor("tri", [P, P], bf, kind="ExternalInput").ap()
    ones = nc.dram_tensor("ones", [P, P], bf, kind="ExternalInput").ap()
    out = nc.dram_tensor("out", [T, D], bf, kind="ExternalOutput").ap()

    xT5 = xT.rearrange("p (c g a t) -> p c g a t", c=NCH, g=NG, a=GA)
    xT4 = xT.rearrange("p (c b t) -> p c b t", c=NCH, b=NB)
    wq4 = wq.rearrange("p (g a c) -> p g a c", g=NG, a=GA)
    wkv5 = wkv.rearrange("p (g w a c) -> p g w a c", g=NG, w=2, a=GA)
    cs5 = cs.rearrange("p (c w k t) -> p c w k t", c=NCH, w=2, k=2)

    with tile.TileContext(nc) as tc, ExitStack() as ctx:
        const = ctx.enter_context(tc.tile_pool(name="const", bufs=1))
        persist = ctx.enter_context(tc.tile_pool(name="persist", bufs=1))
        xt_pool = ctx.enter_context(tc.tile_pool(name="xt", bufs=2))
        cs_pool = ctx.enter_context(tc.tile_pool(name="cs", bufs=2))
        raw_pool = ctx.enter_context(tc.tile_pool(name="raw", bufs=2))
        tmp_pool = ctx.enter_context(tc.tile_pool(name="tmp", bufs=4))
        exp_pool = ctx.enter_context(tc.tile_pool(name="exp", bufs=5))
        rec_pool = ctx.enter_context(tc.tile_pool(name="rec", bufs=2))
        orow_pool = ctx.enter_context(tc.tile_pool(name="orow", bufs=2))

        # Weight/x prologue: interleave (wq_g, xt_g) pairs on the sync
        # ring so the q accumulation chain never outruns its weights;
        # wk/wv/cos/sin stream in parallel on the gpsimd ring, ordered
        # by first use (k matmuls, v matmuls, then RoPE). wo is loaded
        # later, on the sync ring behind the chunk-3 x stream, so it
        # doesn't steal prologue bandwidth.
        wq_sb = [const.tile([P, GA, 256], bf, tag=f"wq{g}", name=f"wq_sb{g}")
                 for g in range(NG)]
        wkv_sb = [const.tile([P, 2, GA, 256], bf, tag=f"wkv{g}",
                             name=f"wkv_sb{g}") for g in range(NG)]
        wk_sb = [wkv_sb[g][:, 0] for g in range(NG)]
        wv_sb = [wkv_sb[g][:, 1] for g in range(NG)]
        xt_c0 = xt_pool.tile([P, NB, TCH], bf, tag="xt", name="xt_0")
        nc.sync.dma_start(wq_sb[0][:], wq4[:, 0])
        nc.sync.dma_start(xt_c0[:, 0:GA, :], xT5[:, 0, 0])
        for g in range(1, NG):
            nc.sync.dma_start(wq_sb[g][:], wq4[:, g])
            nc.sync.dma_start(xt_c0[:, g * GA:(g + 1) * GA, :], xT5[:, 0, g])
        nc.gpsimd.dma_start(wkv_sb[0][:], wkv5[:, 0])
        nc.gpsimd.dma_start(wkv_sb[1][:], wkv5[:, 1])

        cs0 = cs_pool.tile([P, 2, 2, TCH], bf, tag="cs", name="cs_0")
        nc.gpsimd.dma_start(cs0[:], cs5[:, 0])

        for g in (2, 3):
            nc.gpsimd.dma_start(wkv_sb[g][:], wkv5[:, g])

        wo_sb = const.tile([P, 2 * D], bf, tag="wo")
        tri_sb = const.tile([P, P], bf, tag="tri")
        ones_sb = const.tile([P, P], bf, tag="ones")
        nc.gpsimd.dma_start(tri_sb[:], tri[:])
        nc.gpsimd.dma_start(ones_sb[:], ones[:])

        # persistent activations
        q_all = persist.tile([P, 2 * T], bf, tag="q_all")      # roped qT
        k_all = persist.tile([P, 2 * T], bf, tag="k_all")      # roped kT
        v_all = persist.tile([P, 32 * 256], bf, tag="v_all")   # v natural
        at_all = persist.tile([P, 2 * T], bf, tag="at_all")    # attnT

        # ---------------- Phase 1: QKV projections + RoPE ----------------
        with tc.tile_pool(name="psum1", bufs=4, space="PSUM") as psum:
            for tcix in range(NCH - 1):
                t0 = tcix * TCH
                if tcix == 0:
                    xta = lambda a: xt_c0[:, a]
                    cosc, sinc = cs0[:, 0], cs0[:, 1]
                else:
                    xtc = xt_pool.tile([P, NB, TCH], bf, tag="xt",
                                       name=f"xt_{tcix}")
                    nc.sync.dma_start(xtc[:], xT4[:, tcix])
                    xta = lambda a, xtc=xtc: xtc[:, a]
                    csc = cs_pool.tile([P, 2, 2, TCH], bf, tag="cs")
                    nc.sync.dma_start(csc[:], cs5[:, tcix])
                    cosc, sinc = csc[:, 0], csc[:, 1]
                if tcix == 3:
                    # wo behind the chunk-3 x stream on the sync queue:
                    # arrives long before the first output projection.
                    nc.sync.dma_start(wo_sb[:], wo[:])

                qraw = raw_pool.tile([P, 2, TCH], bf, tag="qraw")
                kraw = raw_pool.tile([P, 2, TCH], bf, tag="kraw")
                if tcix == 0:
                    # group-interleaved order: each (wq_g, xt_g, wkv_g)
                    # DMA group unlocks its matmuls immediately, so PE
                    # starts after ~1 group of traffic instead of 3 MB.
                    qk_ps = [psum.tile([P, TCH], f32, tag="qk",
                                       name=f"c0qk{j}") for j in range(4)]
                    v_ps = [psum.tile([P, 256], f32, tag="v",
                                      name=f"c0v{tt}") for tt in range(4)]
                    for g in range(NG):
                        for j, (wt, cb) in enumerate(
                                ((wq_sb, 0), (wq_sb, 1),
                                 (wk_sb, 0), (wk_sb, 1))):
                            for al in range(GA):
                                a = g * GA + al
                                nc.tensor.matmul(
                                    qk_ps[j][:],
                                    wt[g][:, al, cb * P:cb * P + P],
                                    xta(a),
                                    start=(a == 0), stop=(a == NB - 1),
                                )
                        for tt in range(TCH // P):
                            for al in range(GA):
                                a = g * GA + al
                                nc.tensor.matmul(
                                    v_ps[tt][:],
                                    xta(a)[:, tt * P:(tt + 1) * P],
                                    wv_sb[g][:, al, :],
                                    start=(a == 0), stop=(a == NB - 1),
                                )
                    for j, (rawt, cb) in enumerate(
                            ((qraw, 0), (qraw, 1), (kraw, 0), (kraw, 1))):
                        nc.scalar.activation(rawt[:, cb, :], qk_ps[j][:],
                                             Act.Copy)
                    for tt in range(TCH // P):
                        nc.scalar.activation(v_all[:, tt * 256:(tt + 1) * 256],
                                             v_ps[tt][:], Act.Copy)
                else:
                    for (wt, rawt) in ((wq_sb, qraw), (wk_sb, kraw)):
                        for cb in range(2):
                            ps = psum.tile([P, TCH], f32, tag="qk")
                            for a in range(NB):
                                nc.tensor.matmul(
                                    ps[:],
                                    wt[a // GA][:, a % GA,
                                                cb * P:cb * P + P],
                                    xta(a),
                                    start=(a == 0), stop=(a == NB - 1),
                                )
                            nc.scalar.activation(rawt[:, cb, :], ps[:],
                                                 Act.Copy)
                    # v: x-stationary, natural layout
                    for tt in range(TCH // P):
                        ps = psum.tile([P, 256], f32, tag="v")
                        for a in range(NB):
                            nc.tensor.matmul(
                                ps[:],
                                xta(a)[:, tt * P:(tt + 1) * P],
                                wv_sb[a // GA][:, a % GA, :],
                                start=(a == 0), stop=(a == NB - 1),
                            )
                        cidx = (tcix * (TCH // P) + tt) * 256
                        nc.scalar.activation(v_all[:, cidx:cidx + 256], ps[:],
                                             Act.Copy)

                # RoPE: rope_b = raw_b*cos_b + raw_{1-b}*sinT_b (sign-folded)
                for (rawt, dst) in ((qraw, q_all), (kraw, k_all)):
                    for cb in range(2):
                        tm = tmp_pool.tile([P, TCH], bf, tag="ropetmp")
                        nc.vector.tensor_mul(tm[:], rawt[:, 1 - cb, :],
                                             sinc[:, cb, :])
                        tm2 = tmp_pool.tile([P, TCH], bf, tag="ropetmp2")
                        nc.vector.tensor_mul(tm2[:], rawt[:, cb, :],
                                             cosc[:, cb, :])
                        nc.vector.tensor_add(
                            dst[:, cb * T + t0:cb * T + t0 + TCH],
                            tm[:], tm2[:])

        # ------- Phase 2+3: causal attention + output projection -------
        # scoresT blocks [kj=128, q=512]; diagonal blocks shrink to their
        # unmasked column range; exp on ScalarE; per-block causal triangle
        # via a [128,128] GpSimd mask; rowsum via ones-matmul on quad
        # presums (DVE+GpSimd); PV consumes expT directly. The inner loop
        # is software-pipelined two pairs deep (rs/pv trail sc/exp by two
        # pairs) so PE never head-of-line blocks on ScalarE's exp. The
        # output projection for a (b, qj) token group is interleaved one
        # group late, once its at_all slices are long since written.
        def attn_group(psum, b, cb, qj):
            qs = cb * T + b * S + qj * QBLK
            nkb = 4 * qj + 4  # key blocks 0..nkb-1
            npair = nkb // 2
            nquads = nkb // 4
            pv_ps = psum.tile([P, QBLK], f32, tag="pv", bufs=1,
                              name=f"pv_{b}{cb}{qj}")
            rs_ps = psum.tile([P, QBLK], f32, tag="rs", bufs=1,
                              name=f"rs_{b}{cb}{qj}")
            quad_buf = []
            counts = {"pv": 0, "rs": 0}

            def blk_lo(i):
                # first unmasked column of key block i within this q block
                dd = i - 4 * qj
                return 128 * dd if dd > 0 else 0

            def consume(ii, ex):
                for h in range(2):
                    i = 2 * ii + h
                    lo = blk_lo(i)
                    # first matmul of the accumulation must cover the full
                    # bank (start zeroes it); masked ex columns are zero.
                    if counts["pv"] == 0:
                        lo = 0
                    vix = (b * 16 + i) * 256 + cb * P
                    nc.tensor.matmul(pv_ps[:, lo:], v_all[:, vix:vix + P],
                                     ex[:, h, lo:],
                                     start=(counts["pv"] == 0),
                                     stop=(counts["pv"] == nkb - 1))
                    counts["pv"] += 1
                # rowsum: pre-sum 4 exp blocks, one ones-matmul per quad
                quad_buf.append(ex)
                if len(quad_buf) == 2:
                    e0, e1 = quad_buf
                    quad_buf.clear()
                    qi = counts["rs"]
                    # all presums on DVE: gpsimd shares the SBUF port with
                    # DVE, so offloading there slows both engines down.
                    ea = tmp_pool.tile([P, QBLK], bf, tag="esA",
                                       bufs=2, name=f"esA_{b}{cb}{qj}_{qi}")
                    nc.vector.tensor_add(ea[:], e0[:, 0, :], e0[:, 1, :])
                    eb = tmp_pool.tile([P, QBLK], bf, tag="esB",
                                       bufs=2, name=f"esB_{b}{cb}{qj}_{qi}")
                    nc.vector.tensor_add(eb[:], e1[:, 0, :], e1[:, 1, :])
                    es = tmp_pool.tile([P, QBLK], bf, tag="esC",
                                       bufs=2, name=f"esC_{b}{cb}{qj}_{qi}")
                    nc.vector.tensor_add(es[:], ea[:], eb[:])
                    nc.tensor.matmul(rs_ps[:], ones_sb[:], es[:],
                                     start=(qi == 0), stop=(qi == nquads - 1))
                    counts["rs"] += 1

            pending = []

            def pair_step(ii, filler):
                sc_ps = psum.tile([P, 2, QBLK], f32, tag="sc",
                                  name=f"sc_{b}{cb}{qj}_{ii}")
                ex = exp_pool.tile([P, 2, QBLK], bf, tag="exp",
                                   name=f"ex_{b}{cb}{qj}_{ii}")
                los = []
                for h in range(2):
                    i = 2 * ii + h
                    lo = blk_lo(i)
                    los.append(lo)
                    ks = cb * T + b * S + i * P
                    nc.tensor.matmul(sc_ps[:, h, lo:], k_all[:, ks:ks + P],
                                     q_all[:, qs + lo:qs + QBLK],
                                     start=True, stop=True)
                # masked columns of ex must be exact zero for the rowsum
                for h in range(2):
                    if los[h] > 0:
                        nc.gpsimd.memset(ex[:, h, 0:los[h]], 0.0)
                if los[0] == 0 and los[1] == 0:
                    nc.scalar.activation(ex[:], sc_ps[:], Act.Exp,
                                         scale=SCALE)
                else:
                    for h in range(2):
                        nc.scalar.activation(ex[:, h, los[h]:],
                                             sc_ps[:, h, los[h]:],
                                             Act.Exp, scale=SCALE)
                # per-block causal triangle on the diagonal 128 columns
                for h in range(2):
                    i = 2 * ii + h
                    if i >= 4 * qj:
                        lo = los[h]
                        nc.vector.tensor_mul(ex[:, h, lo:lo + P],
                                             ex[:, h, lo:lo + P], tri_sb[:])
                # independent PE work lands here, between the exp issue and
                # the rs/pv matmuls two pairs back (PE executes in order)
                filler()
                pending.append((ii, ex))
                if len(pending) == 3:
                    consume(*pending.pop(0))

            def finish(filler):
                # recip + at-mul issue before the filler's DVE evictions so
                # they sit at the head of the DVE queue: the next group's
                # first pv matmul reuses this pv bank and waits on at-mul.
                while pending:
                    consume(*pending.pop(0))
                rec = rec_pool.tile([P, QBLK], f32, tag="rec",
                                    name=f"rec_{b}{cb}{qj}")
                nc.vector.reciprocal_approx_fast(rec[:], rs_ps[:])
                nc.vector.tensor_mul(at_all[:, qs:qs + QBLK], pv_ps[:], rec[:])
                filler()

            # diagonal pairs first: their exp -> triangle-mask chain then
            # overlaps the dense pairs' matmuls instead of the group tail.
            steps = [(lambda f, ii=ii: pair_step(ii, f))
                     for ii in range(npair - 1, -1, -1)]
            return steps, finish

        def out_units(psum, b, qj, last=False):
            # output projection for the 4 token chunks of (b, qj), split
            # into per-(token, dcol) units so they can fill PE bubbles
            # inside the next attention group's exp-chain.
            units = []

            def unit(tx, dc, orow_box):
                tt = (b * S + qj * QBLK) // P + tx
                if dc == 0:
                    orow_box.append(orow_pool.tile([P, D], bf, tag="orow",
                                                   name=f"orow_{tt}"))
                orow = orow_box[0]
                ps = psum.tile([P, 512], f32, tag="out",
                               name=f"out_{tt}_{dc}")
                for cb in range(2):
                    nc.tensor.matmul(
                        ps[:],
                        at_all[:, cb * T + tt * P:cb * T + (tt + 1) * P],
                        wo_sb[:, cb * D + dc * 512:cb * D + (dc + 1) * 512],
                        start=(cb == 0), stop=(cb == 1),
                    )
                # alternate eviction engine between ACT and DVE
                dst = orow[:, dc * 512:(dc + 1) * 512]
                if dc % 2 == 0:
                    nc.scalar.activation(dst, ps[:], Act.Copy)
                else:
                    nc.vector.tensor_copy(dst, ps[:])
                if dc == D // 512 - 1:
                    # alternate output rows across both DMA rings: halves
                    # the steady-state queue pressure and the tail drain
                    eng = nc.gpsimd if tx % 2 == 1 else nc.sync
                    eng.dma_start(out[tt * P:(tt + 1) * P, :], orow[:])

            for tx in range(QBLK // P):
                box = []
                for dc in range(D // 512):
                    units.append(lambda tx=tx, dc=dc, box=box: unit(tx, dc, box))
            return units

        def chunk7_units(psum):
            # QKV + RoPE for the last token chunk, emitted as PE-filler
            # units inside the b=0 attention section (whose groups don't
            # depend on it). PSUM accumulators borrow the "out" tag slots.
            t0 = (NCH - 1) * TCH
            xt7 = xt_pool.tile([P, NB, TCH], bf, tag="xt", name="xt_7")
            nc.sync.dma_start(xt7[:], xT4[:, NCH - 1])
            cs7 = cs_pool.tile([P, 2, 2, TCH], bf, tag="cs", name="cs_7")
            nc.gpsimd.dma_start(cs7[:], cs5[:, NCH - 1])
            cosc, sinc = cs7[:, 0], cs7[:, 1]
            qraw = raw_pool.tile([P, 2, TCH], bf, tag="qraw", name="qraw_7")
            kraw = raw_pool.tile([P, 2, TCH], bf, tag="kraw", name="kraw_7")

            def qk_unit(wt, rawt, cb, which):
                ps = psum.tile([P, TCH], f32, tag="out",
                               name=f"c7qk_{which}{cb}")
                for a in range(NB):
                    nc.tensor.matmul(
                        ps[:], wt[a // GA][:, a % GA, cb * P:cb * P + P],
                        xt7[:, a],
                        start=(a == 0), stop=(a == NB - 1))
                nc.scalar.activation(rawt[:, cb, :], ps[:], Act.Copy)

            def v_unit(tt):
                ps = psum.tile([P, 256], f32, tag="out", name=f"c7v_{tt}")
                for a in range(NB):
                    nc.tensor.matmul(
                        ps[:], xt7[:, a, tt * P:(tt + 1) * P],
                        wv_sb[a // GA][:, a % GA, :],
                        start=(a == 0), stop=(a == NB - 1))
                cidx = ((NCH - 1) * (TCH // P) + tt) * 256
                nc.scalar.activation(v_all[:, cidx:cidx + 256], ps[:],
                                     Act.Copy)

            def rope_unit(rawt, dst):
                for cb in range(2):
                    tm = tmp_pool.tile([P, TCH], bf, tag="ropetmp")
                    nc.vector.tensor_mul(tm[:], rawt[:, 1 - cb, :],
                                         sinc[:, cb, :])
                    tm2 = tmp_pool.tile([P, TCH], bf, tag="ropetmp2")
                    nc.vector.tensor_mul(tm2[:], rawt[:, cb, :],
                                         cosc[:, cb, :])
                    nc.vector.tensor_add(
                        dst[:, cb * T + t0:cb * T + t0 + TCH], tm[:], tm2[:])

            return [
                lambda: qk_unit(wq_sb, qraw, 0, "q"),
                lambda: qk_unit(wq_sb, qraw, 1, "q"),
                lambda: qk_unit(wk_sb, kraw, 0, "k"),
                lambda: qk_unit(wk_sb, kraw, 1, "k"),
                lambda: v_unit(0), lambda: v_unit(1),
                lambda: v_unit(2), lambda: v_unit(3),
                lambda: rope_unit(qraw, q_all),
                lambda: rope_unit(kraw, k_all),
            ]

        with tc.tile_pool(name="psum2", bufs=2, space="PSUM") as psum:
            groups = [(b, qj) for b in range(B) for qj in range(NQ)]
            c7 = chunk7_units(psum)
            noop = lambda: None
            # each group's finish is delayed two pair-steps into the next
            # group: its pv/rs psum chain (recip, at-mul on DVE) then hides
            # behind the next group's score matmuls instead of stalling PE
            # at every group boundary (pv/rs have a single psum bank).
            fin_pend = []
            for gi, (b, qj) in enumerate(groups):
                emits = []
                for cb in range(2):
                    psteps, fin = attn_group(psum, b, cb, qj)
                    for si, st in enumerate(psteps):
                        emits.append(st)
                        if si == 1 and fin_pend:
                            emits.append(fin_pend.pop(0))
                    fin_pend.append(fin)
                outs = out_units(psum, *groups[gi - 1]) if gi >= 1 else []
                if gi < 4:  # spread chunk-7 work over the b=0 groups
                    outs = outs + c7[gi * 3:min((gi + 1) * 3, len(c7))]
                # fillers start at position 4: after the previous group's
                # delayed finish (position 2) has written its at_all slice.
                k = 0
                n = len(emits)
                for ei, fn in enumerate(emits):
                    if ei < 4 or not outs:
                        fn(noop)
                        continue
                    tgt = (ei - 3) * len(outs) // (n - 4)

                    def filler(tgt=tgt, outs=outs):
                        nonlocal k
                        while k < tgt:
                            outs[k]()
                            k += 1
                    fn(filler)
                # flush any remainder of this gi's filler budget
                while outs and k < len(outs):
                    outs[k]()
                    k += 1
            for fin in fin_pend:
                fin(noop)
            for u in out_units(psum, *groups[-1], last=True):
                u()

    nc.compile()
    return nc


def _host_prep(x, cos, sin, Wq, Wk, Wv, Wo):
    """Build per-core input maps (numpy, bf16 on-device dtypes)."""
    def pblock(arr, nblk):
        # [nblk*128, F] -> [128, nblk*F] with col = a*F + f
        nb, f = nblk, arr.shape[1]
        return np.ascontiguousarray(
            arr.reshape(nb, P, f).transpose(1, 0, 2).reshape(P, nb * f))

    x2 = np.asarray(x, np.float32).reshape(T, D)
    # chunk-major x: col = ((c*NG + g)*GA + al)*TCH + tl so each chunk's
    # per-partition row is one contiguous 16 KB run in DRAM.
    xcore = np.ascontiguousarray(x2.T)                  # [D, T]
    xr = xcore.reshape(NG, GA, P, NCH, TCH)
    xT_r = np.ascontiguousarray(
        xr.transpose(2, 3, 0, 1, 4).reshape(P, NCH * NB * TCH)).astype(BF16)

    cosn = np.asarray(cos, np.float32)
    sinn = np.asarray(sin, np.float32)
    Wqn = np.asarray(Wq, np.float32)
    Wkn = np.asarray(Wk, np.float32)
    Wvn = np.asarray(Wv, np.float32)
    Won = np.asarray(Wo, np.float32)

    # intra-block causal triangle: tri[kj, qq] = (qq >= kj)
    kj = np.arange(P)[:, None]
    qq = np.arange(P)[None, :]
    tri = (qq >= kj).astype(np.float32)

    common = {
        "xT": xT_r,
        "tri": tri.astype(BF16),
        "ones": np.ones((P, P), BF16),
    }

    in_maps = []
    for m in range(N_CORES):
        cols = np.r_[128 * m:128 * m + 128, 1024 + 128 * m:1024 + 128 * m + 128]
        wq_s = pblock(Wqn[:, cols], NB).astype(BF16)
        wk_s = pblock(Wkn[:, cols], NB).astype(BF16)
        wv_s = pblock(Wvn[:, cols], NB).astype(BF16)
        wo_s = pblock(Won[cols, :], 2).astype(BF16)
        # merged k/v weights: col = ((g*2 + w)*GA + al)*256 + c
        wkv_s = np.ascontiguousarray(
            np.stack([wk_s.reshape(P, NG, GA * 256),
                      wv_s.reshape(P, NG, GA * 256)], axis=2)
            .reshape(P, NG * 2 * GA * 256))

        ct = np.tile(cosn[:, cols].T, (1, B))          # [256, 4096]
        st = np.tile(sinn[:, cols].T, (1, B)).copy()
        st[:128] *= -1.0                               # sign-fold block0
        # merged chunk-major cos/sin: [p, c, w(cos/sin), k(cb), tl]
        c5 = ct.reshape(2, P, NCH, TCH).transpose(1, 2, 0, 3)
        s5 = st.reshape(2, P, NCH, TCH).transpose(1, 2, 0, 3)
        cs_s = np.ascontiguousarray(
            np.stack([c5, s5], axis=2).reshape(P, NCH * 4 * TCH)).astype(BF16)

        in_maps.append(dict(common, wq=wq_s, wkv=wkv_s.astype(BF16),
                            wo=wo_s, cs=cs_s))
    return in_maps


def _numpy_fallback(x, cos, sin, Wq, bq, Wk, bk, Wv, bv, Wo, bo):
    """Exact fp32 reference path (only used when bq/bk are nonzero,
    which the spec's zero-filled biases never trigger)."""
    b, s, d = x.shape
    x2 = np.asarray(x, np.float32)
    q = x2 @ Wq + bq
    k = x2 @ Wk + bk
    v = x2 @ Wv + bv

    def rope(t):
        neg = np.concatenate([-t[..., d // 2:], t[..., :d // 2]], axis=-1)
        return t * cos[:s] + neg * sin[:s]

    q = rope(q).reshape(b, s, H, HD)
    k = rope(k).reshape(b, s, H, HD)
    v = v.reshape(b, s, H, HD)
    sc = np.einsum('bqhd,bkhd->bhqk', q, k) / np.sqrt(HD)
    mask = np.tril(np.ones((s, s), bool))
    sc = np.where(mask, sc, -np.inf)
    sc -= sc.max(-1, keepdims=True)
    p = np.exp(sc)
    p /= p.sum(-1, keepdims=True)
    at = np.einsum('bhqk,bkhd->bqhd', p, v).reshape(b, s, d)
    return at @ Wo + bo


def kernel(x, cos, sin, Wq, bq, Wk, bk, Wv, bv, Wo, bo):
    global LAST_RESULTS
    from concourse.bass_utils import run_bass_kernel_spmd

    if np.any(np.asarray(bq)) or np.any(np.asarray(bk)):
        return _numpy_fallback(x, cos, sin,
                               np.asarray(Wq, np.float32), np.asarray(bq, np.float32),
                               np.asarray(Wk, np.float32), np.asarray(bk, np.float32),
                               np.asarray(Wv, np.float32), np.asarray(bv, np.float32),
                               np.asarray(Wo, np.float32), np.asarray(bo, np.float32))

    if "nc" not in _CACHE:
        _CACHE["nc"] = _build_program()
    nc = _CACHE["nc"]

    in_maps = _host_prep(x, cos, sin, Wq, Wk, Wv, Wo)
    res = run_bass_kernel_spmd(nc, in_maps, core_ids=list(range(N_CORES)))
    LAST_RESULTS = res

    acc = np.zeros((T, D), np.float32)
    for r in res.results:
        acc += r["out"].astype(np.float32)
    # v-bias and output bias: attn rows sum to 1, so bv contributes bv @ Wo.
    acc += (np.asarray(bv, np.float32) @ np.asarray(Wo, np.float32)
            + np.asarray(bo, np.float32))[None, :]
    return acc.reshape(B, S, D)
